# revision 15
# baseline (speedup 1.0000x reference)
"""CKANKANNet Trainium2 kernel builder v2 (per-core SPMD program, B=8 samples/core).

Algorithm: v = 2.5*x + 5.5 maps the spline grid to integer knots. The
conv spline path consumes 3rd-difference slabs d3_s (s=0..8) of the
relu-cube chain (x clamped at 2.2), with the 4th difference folded into
the conv weights: W'_s = (w_s - w_{s-1})/6. d3 values lie in [0, 6.0004]
so fp16 storage is safe (validated offline: 6e-5 abs err).

Engine split per basis chunk: ACT computes RX + 12 shifted relus + one
batched Square; DVE computes cube mult, d1, d3; Pool (gpsimd) computes d2.
Layers run in image-pair sub-batches with double-buffered tiles and PSUM
so basis(k+1) overlaps conv(k).

L1 pairs images (b, b+4) in one 128-partition PSUM tile (img b at
partitions 0:64, b+4 at 64:128) so maxpool runs 128-lane and writes the
L2-layout h1s directly. L3 needs no scatter at all: d3 ops write
strided straight into the conv moving tile (128 channels = 128 partitions).
"""
import sys
sys.path.insert(0, '/opt/trn_rl_repo')
from contextlib import ExitStack

import numpy as np

MM_NP = np.float16

import concourse.bass as bass
import concourse.tile as tile
from concourse import bacc, mybir

F32 = mybir.dt.float32
F16 = mybir.dt.float16
MMDT = mybir.dt.float16
AF = mybir.ActivationFunctionType
OP = mybir.AluOpType

B = 8
NB = 8      # basis functions per activation
NS = 9      # d3 slabs per activation (d4 folded into weights)
NM = 12     # relu-cube slabs
O_OUT = 100
E = 256     # basis chunk width


def fold_weights(wb1, ws1, wb2, ws2, wb3, ws3, lb, lc):
    """Fold the 4th finite difference into conv weights:
    W'_s = (w_s - w_{s-1})/6 for s=0..8 (w_{-1} = w_{8} = 0)."""
    out = {}

    def d4fold(ws_k):  # ws_k: [O, C, NB] -> [NS, C, O]
        O_, C_, _ = ws_k.shape
        f = np.zeros((NS, C_, O_), np.float32)
        wt = np.transpose(ws_k, (2, 1, 0)) / 6.0  # [NB, C, O]
        f[0:NB] += wt
        f[1:NS] -= wt
        return f

    # W1: [90, 192]; per-ky 30-row block: rows 0:3 silu, 3:30 = 9 slabs x 3ch
    W1 = np.zeros((90, 3 * 64), np.float32)
    for kyi in range(3):
        for kxi in range(3):
            W1[kyi * 30 + 0:kyi * 30 + 3, kxi * 64:(kxi + 1) * 64] = wb1[:, :, kyi, kxi].T
            blk = d4fold(ws1[:, :, kyi, kxi].reshape(64, 3, NB))  # [9, 3, 64]
            W1[kyi * 30 + 3:kyi * 30 + 30, kxi * 64:(kxi + 1) * 64] = blk.reshape(27, 64)
    out['w1'] = W1.astype(MM_NP)

    # W2: [640, 1152]; rows 0:576 = 9 slabs x 64ch (slab-major), 576:640 silu
    W2 = np.zeros((640, 9 * 128), np.float32)
    for kyi in range(3):
        for kxi in range(3):
            t = kyi * 3 + kxi
            blk = d4fold(ws2[:, :, kyi, kxi].reshape(128, 64, NB))  # [9, 64, 128]
            W2[0:576, t * 128:(t + 1) * 128] = blk.reshape(576, 128)
            W2[576:640, t * 128:(t + 1) * 128] = wb2[:, :, kyi, kxi].T
    out['w2'] = W2.astype(MM_NP)

    # W3: [1280, 576]; rows 0:1152 = 9 slabs x 128ch, 1152:1280 silu
    W3 = np.zeros((1280, 9 * 64), np.float32)
    for kyi in range(3):
        for kxi in range(3):
            t = kyi * 3 + kxi
            blk = d4fold(ws3[:, :, kyi, kxi].reshape(64, 128, NB))  # [9, 128, 64]
            W3[0:1152, t * 64:(t + 1) * 64] = blk.reshape(1152, 64)
            W3[1152:1280, t * 64:(t + 1) * 64] = wb3[:, :, kyi, kxi].T
    out['w3'] = W3.astype(MM_NP)

    # WL: [128, 32*9*100]; rows (c + 64*par), col ((p2*9 + j)*100 + o)
    # j=0..7 spline basis (d4 stays on DVE for the linear layer), j=8 silu.
    # yx = 2*p2 + par.
    lc_r = lc.reshape(O_OUT, 64, 64, NB)
    lb_r = lb.reshape(O_OUT, 64, 64)
    WL = np.zeros((2, 64, 32, 9, O_OUT), np.float32)
    for par in range(2):
        for p2 in range(32):
            yx = 2 * p2 + par
            for j in range(NB):
                WL[par, :, p2, j, :] = lc_r[:, :, yx, j].T / 6.0
            WL[par, :, p2, 8, :] = lb_r[:, :, yx].T
    out['wl'] = WL.reshape(128, 32 * 9 * O_OUT).astype(MM_NP)
    return out


def emit_basis(nc, bp, src_ap, P, d3_out_3d, bias_tiles, Ec=E, d2_pool=True):
    """Basis chunk: src [P, Ec] activations -> d3 slabs [P, NS, Ec] fp16 (3D AP).

    Engine split: ACT: RX, 12x T, 1x Square; DVE: cube, d1, d3; Pool: d2
    (d2_pool=False puts d2 on DVE: lower latency for serial-tail chunks).
    """
    RX = bp.tile([P, Ec], F32, tag="rx")
    nc.scalar.activation(RX[:], src_ap, AF.Relu, bias=bias_tiles['c22'][0:P, :],
                         scale=-1.0)
    T12 = bp.tile([P, NM * Ec], F32, tag="t12")
    for m in range(NM):
        nc.scalar.activation(T12[:, m * Ec:(m + 1) * Ec], RX[:], AF.Relu,
                             bias=bias_tiles[m][0:P, :], scale=-2.5)
    S12 = bp.tile([P, NM * Ec], F32, tag="s12")
    nc.scalar.activation(S12[:], T12[:], AF.Square)
    C3 = bp.tile([P, NM * Ec], F32, tag="c3")
    nc.vector.tensor_tensor(C3[:], T12[:], S12[:], op=OP.mult)
    D1 = bp.tile([P, (NM - 1) * Ec], F32, tag="t12", name="D1")
    nc.vector.tensor_tensor(D1[:], C3[:, 0:(NM - 1) * Ec], C3[:, Ec:NM * Ec], op=OP.subtract)
    D2 = bp.tile([P, (NM - 2) * Ec], F32, tag="s12", name="D2")
    eng = nc.gpsimd if d2_pool else nc.vector
    eng.tensor_tensor(D2[:], D1[:, 0:(NM - 2) * Ec], D1[:, Ec:(NM - 1) * Ec], op=OP.subtract)
    nc.vector.tensor_tensor(
        d3_out_3d,
        D2[:, 0:NS * Ec].rearrange("p (s e) -> p s e", e=Ec),
        D2[:, Ec:(NS + 1) * Ec].rearrange("p (s e) -> p s e", e=Ec),
        op=OP.subtract)


def build_nc(dbg=()):
    nc = bacc.Bacc("TRN2", target_bir_lowering=False, debug=False, num_devices=8)
    x_ext = nc.declare_dram_parameter("x", [B, 3, 64, 64], F32, isOutput=False)
    w1_ext = nc.declare_dram_parameter("w1", [90, 192], MMDT, isOutput=False)
    w2_ext = nc.declare_dram_parameter("w2", [640, 1152], MMDT, isOutput=False)
    w3_ext = nc.declare_dram_parameter("w3", [1280, 576], MMDT, isOutput=False)
    wl_ext = nc.declare_dram_parameter("wl", [128, 28800], MMDT, isOutput=False)
    out_ext = nc.declare_dram_parameter("out", [B, O_OUT], F32, isOutput=True)

    dbg_exts = {}

    def dbg_tap(name, shape, dt=F32):
        if name in dbg:
            dbg_exts[name] = nc.declare_dram_parameter(f"dbg_{name}", shape, dt, isOutput=True)
            return dbg_exts[name]
        return None

    with tile.TileContext(nc) as tc, ExitStack() as ctx:
        persist = ctx.enter_context(tc.tile_pool(name="persist", bufs=1))
        wpool = ctx.enter_context(tc.tile_pool(name="wpool", bufs=1))
        bpool = ctx.enter_context(tc.tile_pool(name="bpool", bufs=2))

        bias_tiles = {}
        for m in range(NM):
            bt_m = wpool.tile([128, 1], F32, tag=f"bias_{m}", name=f"bias{m}")
            nc.gpsimd.memset(bt_m[:], float(11 - m))
            bias_tiles[m] = bt_m
        bt_c = wpool.tile([128, 1], F32, tag="bias_c22", name="biasc22")
        nc.gpsimd.memset(bt_c[:], 2.2)
        bias_tiles['c22'] = bt_c

        h1s = persist.tile([128, 4096], F16)   # L2 input: (ch, b<4|b>=4) x (sb, px)
        h2 = persist.tile([128, 2048], F32)    # L3 input: ch x (b, px)
        h3 = persist.tile([64, 512], F32)      # linear input: ch x (b, yx)

        # ================= L1 =================
        l1ch = nc.dram_tensor("l1ch", [30, B * 4096], MMDT)
        w1sb = wpool.tile([90, 192], MMDT)
        w2sb = wpool.tile([128, 5 * 1152], MMDT)
        w3sb = wpool.tile([128, 10 * 576], MMDT)
        zt = wpool.tile([30, 64], MMDT)
        with tc.tile_pool(name="l1pool", bufs=1) as l1p:
            # input + w1 first so the basis chain starts ASAP
            X1 = l1p.tile([128, 768], F32)
            for c in range(3):
                nc.sync.dma_start(
                    X1[:, c * 256:(c + 1) * 256],
                    x_ext.ap()[:, c, :, :].rearrange("b (g hh) w -> b g (hh w)", g=16))
            nc.sync.dma_start(w1sb[:], w1_ext.ap())
            nc.gpsimd.memset(zt[:], 0.0)

            sl1 = l1p.tile([128, 768], MMDT)
            nc.scalar.activation(sl1[:], X1[:], AF.Silu)

            D3_1 = l1p.tile([128, NS * 768], MMDT)
            d31_v = D3_1[:].rearrange("p (s e) -> p s e", s=NS)
            for ck in range(3):
                emit_basis(nc, bpool, X1[:, ck * E:(ck + 1) * E], 128,
                           d31_v[:, :, ck * E:(ck + 1) * E], bias_tiles,
                           d2_pool=False)

            # stage to channel-major DRAM: rows 0:3 silu ch, 3+s*3+c for slabs
            for s in range(NS):
                nc.sync.dma_start(
                    l1ch.ap()[3 + s * 3:3 + s * 3 + 3, :]
                        .rearrange("c (bg e) -> bg c e", e=256),
                    D3_1[:, s * 768:(s + 1) * 768]
                        .rearrange("p (c e) -> p c e", e=256))
            for c in range(3):
                nc.scalar.dma_start(
                    l1ch.ap()[c, :].rearrange("(bg e) -> bg e", e=256),
                    sl1[:, c * 256:(c + 1) * 256])
            # prefetch L2/L3 weights on the idle ACT queue
            for i in range(5):
                nc.scalar.dma_start(w2sb[:, i * 1152:(i + 1) * 1152],
                                    w2_ext.ap()[i * 128:(i + 1) * 128, :])
            for i in range(10):
                nc.scalar.dma_start(w3sb[:, i * 576:(i + 1) * 576],
                                    w3_ext.ap()[i * 128:(i + 1) * 128, :])

            # per image-pair (sb, sb+4) buns; img sb at bun pos 0 -> psum 0:64
            with tc.tile_pool(name="l1bun", bufs=2) as lbp, \
                    tc.tile_pool(name="pp1", bufs=2, space="PSUM") as pp1:
                for sb in range(4):
                    Bun1 = lbp.tile([90, 64 + 2 * 4096 + 64], MMDT, tag="bun1",
                                    name=f"Bun1_{sb}")
                    for kyi in range(3):
                        base = 64 + (1 - kyi) * 64
                        nc.sync.dma_start(
                            Bun1[kyi * 30:kyi * 30 + 30, base:base + 2 * 4096]
                                .rearrange("p (b2 e) -> p b2 e", b2=2),
                            l1ch.ap()[:, :].rearrange("p (b e) -> p b e", b=8)[
                                :, sb::4, :])
                    # boundary rows: ky=0 block box-row 0; ky=2 block box-row 63
                    for bi in range(2):
                        nc.sync.dma_start(
                            Bun1[0:30, 64 + bi * 4096: 64 + bi * 4096 + 64], zt[:])
                        nc.sync.dma_start(
                            Bun1[60:90, 64 + bi * 4096 + 63 * 64: 64 + bi * 4096 + 64 * 64],
                            zt[:])

                    bun1_v = Bun1[:, 64:64 + 2 * 4096].rearrange(
                        "p (b r w) -> p b r w", b=2, w=64)
                    for grp in range(2):
                        pss = [pp1.tile([128, 512], F32, tag=f"ps1_{i}",
                                        name=f"ps1_{sb}_{grp}_{i}") for i in range(4)]
                        for ti, kxi in enumerate([1, 0, 2]):
                            for ci in range(4):
                                hb = grp * 4 + ci
                                ps = pss[ci]
                                for half in range(2):
                                    if kxi == 0:
                                        mv = bun1_v[:, half, hb * 8:hb * 8 + 8, 0:63]
                                        ov = ps[half * 64:half * 64 + 64, :].rearrange(
                                            "p (r w) -> p r w", w=64)[:, :, 1:64]
                                    elif kxi == 1:
                                        mv = bun1_v[:, half, hb * 8:hb * 8 + 8, :]
                                        ov = ps[half * 64:half * 64 + 64, :]
                                    else:
                                        mv = bun1_v[:, half, hb * 8:hb * 8 + 8, 1:64]
                                        ov = ps[half * 64:half * 64 + 64, :].rearrange(
                                            "p (r w) -> p r w", w=64)[:, :, 0:63]
                                    nc.tensor.matmul(ov, w1sb[:, kxi * 64:(kxi + 1) * 64],
                                                     mv, start=(ti == 0), stop=(ti == 2))
                        for ci in range(4):
                            hb = grp * 4 + ci
                            pv = pss[ci][:].rearrange("p (hp r2 wp c2) -> p hp wp r2 c2",
                                                      hp=4, r2=2, wp=32, c2=2)
                            nc.vector.tensor_reduce(
                                h1s[:, sb * 1024 + hb * 128: sb * 1024 + (hb + 1) * 128]
                                    .rearrange("p (hp wp) -> p hp wp", wp=32),
                                pv, mybir.AxisListType.XY, OP.max, opt_input=False)
        if (t := dbg_tap('h1s', [128, 4096], F16)) is not None:
            nc.sync.dma_start(t.ap(), h1s[:])

        # ================= L2 =================
        # sub-batch sb: images (sb, sb+4) = h1s cols [sb*1024, (sb+1)*1024)
        with tc.tile_pool(name="l2t", bufs=2) as l2t, \
                tc.tile_pool(name="l2s", bufs=2) as l2s, \
                tc.tile_pool(name="pp2", bufs=2, space="PSUM") as pp2:
            for sb in range(4):
                T2 = l2t.tile([128, 5 * 2048], MMDT, tag="t2", name=f"T2_{sb}")
                # silu: img sb+4 (h1s rows 64:128) writes lanes 64:128 directly
                nc.scalar.activation(T2[64:128, 4 * 2048 + 1024:4 * 2048 + 2048],
                                     h1s[64:128, sb * 1024:(sb + 1) * 1024], AF.Silu)
                Ts2a = l2s.tile([64, 1024], MMDT, tag="ts2a", name=f"Ts2a_{sb}")
                nc.scalar.activation(Ts2a[:], h1s[0:64, sb * 1024:(sb + 1) * 1024], AF.Silu)
                nc.sync.dma_start(T2[64:128, 4 * 2048:4 * 2048 + 1024], Ts2a[:])

                D3b = l2s.tile([128, NS * 1024], MMDT, tag="d3", name=f"D3_{sb}")
                d3v = D3b[:].rearrange("p (s q) -> p s q", s=NS)
                for ck in range(4):
                    emit_basis(nc, bpool,
                               h1s[:, sb * 1024 + ck * E: sb * 1024 + (ck + 1) * E],
                               128, d3v[:, :, ck * E:(ck + 1) * E], bias_tiles)
                # scatter into T2: slab s -> col-block s//2, rows (s%2)*64+c,
                # col ph*1024 (6 batched DMAs per sub-batch)
                for ph in range(2):
                    for s1 in range(2):
                        nc.sync.dma_start(
                            T2[s1 * 64:s1 * 64 + 64, :]
                                .rearrange("p (s2 q) -> p s2 q", s2=5)[
                                    :, 0:4, ph * 1024:(ph + 1) * 1024],
                            d3v[ph * 64:(ph + 1) * 64, s1:8:2, :])
                    nc.sync.dma_start(
                        T2[0:64, 4 * 2048 + ph * 1024:4 * 2048 + (ph + 1) * 1024],
                        d3v[ph * 64:(ph + 1) * 64, 8, :])

                t2v = T2[:].rearrange("p (kt b h w) -> p kt b h w", kt=5, b=2, w=32)
                taps = [(0, 1, 1)] + [(kt, kyi, kxi) for kt in range(5)
                                      for kyi in range(3) for kxi in range(3)
                                      if (kt, kyi, kxi) != (0, 1, 1)]
                n_taps = len(taps)
                pss = [pp2.tile([128, 512], F32, tag=f"ps2_{i}", name=f"ps2_{sb}_{i}")
                       for i in range(4)]
                for tapi, (kt, kyi, kxi) in enumerate(taps):
                    for ci in range(4):
                        ii, half = ci // 2, ci % 2
                        h0 = half * 16
                        ps = pss[ci]
                        r_lo = max(0, 1 - kyi - h0)
                        r_hi = min(16, 33 - h0 - kyi)
                        w_lo = 1 if kxi == 0 else 0
                        w_hi = 31 if kxi == 2 else 32
                        in_row = h0 + r_lo + kyi - 1
                        in_col = w_lo + kxi - 1
                        mv = t2v[:, kt, ii, in_row:in_row + (r_hi - r_lo),
                                 in_col:in_col + (w_hi - w_lo)]
                        ov = ps[:].rearrange("p (r w) -> p r w", w=32)[
                            :, r_lo:r_hi, w_lo:w_hi]
                        nc.tensor.matmul(
                            ov, w2sb[:, kt * 1152 + (kyi * 3 + kxi) * 128:
                                     kt * 1152 + (kyi * 3 + kxi + 1) * 128],
                            mv, start=(tapi == 0), stop=(tapi == n_taps - 1))
                for ci in range(4):
                    ii, half = ci // 2, ci % 2
                    b = sb + 4 * ii
                    pv = pss[ci][:].rearrange("p (hp r2 wp c2) -> p hp wp r2 c2",
                                              hp=8, r2=2, wp=16, c2=2)
                    nc.vector.tensor_reduce(
                        h2[:, b * 256 + half * 128: b * 256 + (half + 1) * 128]
                            .rearrange("p (hp wp) -> p hp wp", wp=16),
                        pv, mybir.AxisListType.XY, OP.max, opt_input=False)
        if (t := dbg_tap('h2', [128, 2048])) is not None:
            nc.sync.dma_start(t.ap(), h2[:])

        # prefetch wl during L3 (pool opened here: reuses bytes freed by L1/L2)
        wlp = ctx.enter_context(tc.tile_pool(name="wlpool", bufs=1))
        wlsb = wlp.tile([128, 28800], MMDT)
        for i in range(4):
            nc.sync.dma_start(wlsb[:, i * 7200:(i + 1) * 7200],
                              wl_ext.ap()[:, i * 7200:(i + 1) * 7200])

        # ================= L3 (+ interleaved linear basis halves) ==========
        # linear: h3r rows (c + 64*par), cols (b, y2); half hf = imgs 4hf..4hf+3
        h3r = persist.tile([128, 256], F32)
        sl3 = persist.tile([128, 256], MMDT)
        D3L = persist.tile([128, NS * 256], MMDT)
        D4L = persist.tile([128, NB * 256], MMDT)
        h3v = h3[:].rearrange("p (b y2 par) -> p b y2 par", b=8, par=2)
        d3lv = D3L[:].rearrange("p (s q) -> p s q", s=NS)

        def emit_lin_half(hf):
            for par in range(2):
                nc.sync.dma_start(
                    h3r[par * 64:par * 64 + 64, hf * 128:(hf + 1) * 128]
                        .rearrange("p (b y2) -> p b y2", b=4),
                    h3v[:, 4 * hf:4 * hf + 4, :, par])
            nc.scalar.activation(sl3[:, hf * 128:(hf + 1) * 128],
                                 h3r[:, hf * 128:(hf + 1) * 128], AF.Silu)
            emit_basis(nc, bpool, h3r[:, hf * 128:(hf + 1) * 128], 128,
                       d3lv[:, :, hf * 128:(hf + 1) * 128],
                       bias_tiles, Ec=128, d2_pool=False)
            nc.vector.tensor_tensor(
                D4L[:].rearrange("p (j q) -> p j q", j=NB)[:, :, hf * 128:(hf + 1) * 128],
                d3lv[:, 0:NB, hf * 128:(hf + 1) * 128],
                d3lv[:, 1:NS, hf * 128:(hf + 1) * 128],
                op=OP.subtract)

        # sub-batch sb: images (2sb, 2sb+1) = h2 cols [sb*512, (sb+1)*512)
        with tc.tile_pool(name="l3t", bufs=2) as l3t, \
                tc.tile_pool(name="pp3", bufs=2, space="PSUM") as pp3:
            for sb in range(4):
                T3 = l3t.tile([128, 10 * 512], MMDT, tag="t3", name=f"T3_{sb}")
                for ph in range(2):
                    b = 2 * sb + ph
                    nc.scalar.activation(T3[:, 9 * 512 + ph * 256:9 * 512 + (ph + 1) * 256],
                                         h2[:, b * 256:(b + 1) * 256], AF.Silu)
                    # d3 writes strided straight into T3 slab blocks
                    emit_basis(nc, bpool, h2[:, b * 256:(b + 1) * 256], 128,
                               T3[:].rearrange("p (s e) -> p s e", e=512)[
                                   :, 0:NS, ph * 256:(ph + 1) * 256],
                               bias_tiles)

                t3v = T3[:].rearrange("p (kt b h w) -> p kt b h w", kt=10, b=2, w=16)
                taps = [(0, 1, 1)] + [(kt, kyi, kxi) for kt in range(10)
                                      for kyi in range(3) for kxi in range(3)
                                      if (kt, kyi, kxi) != (0, 1, 1)]
                n_taps = len(taps)
                ps = pp3.tile([64, 512], F32, tag="ps3", name=f"ps3_{sb}")
                for tapi, (kt, kyi, kxi) in enumerate(taps):
                    r_lo = max(0, 1 - kyi)
                    r_hi = min(16, 17 - kyi)
                    w_lo = 1 if kxi == 0 else 0
                    w_hi = 15 if kxi == 2 else 16
                    mv = t3v[:, kt, :, r_lo + kyi - 1:r_hi + kyi - 1,
                             w_lo + kxi - 1:w_lo + kxi - 1 + (w_hi - w_lo)]
                    ov = ps[:].rearrange("p (b r w) -> p b r w", b=2, w=16)[
                        :, :, r_lo:r_hi, w_lo:w_hi]
                    nc.tensor.matmul(
                        ov, w3sb[:, kt * 576 + (kyi * 3 + kxi) * 64:
                                 kt * 576 + (kyi * 3 + kxi + 1) * 64],
                        mv, start=(tapi == 0), stop=(tapi == n_taps - 1))
                pv = ps[:].rearrange("p (b hp r2 wp c2) -> p b hp wp r2 c2",
                                     b=2, hp=8, r2=2, wp=8, c2=2)
                for ph in range(2):
                    b = 2 * sb + ph
                    nc.vector.tensor_reduce(
                        h3[:, b * 64:(b + 1) * 64].rearrange("p (hp wp) -> p hp wp", wp=8),
                        pv[:, ph], mybir.AxisListType.XY, OP.max, opt_input=False)
                if sb == 1:
                    emit_lin_half(0)   # overlaps L3 sb2/sb3 conv
            emit_lin_half(1)
        if (t := dbg_tap('h3', [64, 512])) is not None:
            nc.sync.dma_start(t.ap(), h3[:])
        if (t := dbg_tap('d4l', [128, NB * 256], F16)) is not None:
            nc.sync.dma_start(t.ap(), D4L[:])
        if (t := dbg_tap('sl3', [128, 256], F16)) is not None:
            nc.sync.dma_start(t.ap(), sl3[:])
        if (t := dbg_tap('h3r', [128, 256])) is not None:
            nc.sync.dma_start(t.ap(), h3r[:])

        # ================= Linear matmuls =================
        with tc.tile_pool(name="ppl", bufs=1, space="PSUM") as plin:
            psl = plin.tile([B, O_OUT], F32)
            d4l_v = D4L[:].rearrange("p (j b y2) -> p j b y2", j=NB, b=B)
            sl3_v = sl3[:].rearrange("p (b y2) -> p b y2", b=B)
            for p2 in range(32):
                for j in range(NB):
                    nc.tensor.matmul(
                        psl[:], d4l_v[:, j, :, p2],
                        wlsb[:, (p2 * 9 + j) * O_OUT:(p2 * 9 + j + 1) * O_OUT],
                        start=(p2 == 0 and j == 0), stop=False)
                nc.tensor.matmul(
                    psl[:], sl3_v[:, :, p2],
                    wlsb[:, (p2 * 9 + 8) * O_OUT:(p2 * 9 + 9) * O_OUT],
                    start=False, stop=(p2 == 31))
            osb = persist.tile([B, O_OUT], F32)
            nc.vector.tensor_copy(osb[:], psl[:])
            nc.sync.dma_start(out_ext.ap(), osb[:])

    nc.compile()
    return nc

# ===================================================================== runner
from concourse.bass_utils import run_bass_kernel_spmd

_NC_CACHE = {}


def _get_nc():
    if 'nc' not in _NC_CACHE:
        _NC_CACHE['nc'] = build_nc(dbg=())
    return _NC_CACHE['nc']


def kernel(x, wb1, ws1, wb2, ws2, wb3, ws3, lb, lc):
    """Full-input entry point: x [64,3,64,64] f32 -> out [64,100] f32.
    Shards the batch over 8 NeuronCores (8 samples each), replicating weights."""
    x = np.ascontiguousarray(np.asarray(x, dtype=np.float32))
    w = fold_weights(np.asarray(wb1, np.float32), np.asarray(ws1, np.float32),
                     np.asarray(wb2, np.float32), np.asarray(ws2, np.float32),
                     np.asarray(wb3, np.float32), np.asarray(ws3, np.float32),
                     np.asarray(lb, np.float32), np.asarray(lc, np.float32))
    nc = _get_nc()
    in_maps = [{'x': x[i * B:(i + 1) * B], **w} for i in range(8)]
    res = run_bass_kernel_spmd(nc, in_maps, core_ids=list(range(8)))
    return np.concatenate([res.results[i]['out'] for i in range(8)], axis=0)


# revision 23
# speedup vs baseline: 1.0394x; 1.0394x over previous
"""CKANKANNet Trainium2 kernel builder v2 (per-core SPMD program, B=8 samples/core).

Algorithm: v = 2.5*x + 5.5 maps the spline grid to integer knots. The
conv spline path consumes 3rd-difference slabs d3_s (s=0..8) of the
relu-cube chain (x clamped at 2.2), with the 4th difference folded into
the conv weights: W'_s = (w_s - w_{s-1})/6. d3 values lie in [0, 6.0004]
so fp16 storage is safe (validated offline: 6e-5 abs err).

Engine split per basis chunk: ACT computes RX + 12 shifted relus + one
batched Square; DVE computes cube mult, d1, d3; Pool (gpsimd) computes d2.
Layers run in image-pair sub-batches with double-buffered tiles and PSUM
so basis(k+1) overlaps conv(k).

L1 pairs images (b, b+4) in one 128-partition PSUM tile (img b at
partitions 0:64, b+4 at 64:128) so maxpool runs 128-lane and writes the
L2-layout h1s directly. L3 needs no scatter at all: d3 ops write
strided straight into the conv moving tile (128 channels = 128 partitions).
"""
import sys
sys.path.insert(0, '/opt/trn_rl_repo')
from contextlib import ExitStack

import numpy as np

MM_NP = np.float16

import concourse.bass as bass
import concourse.tile as tile
from concourse import bacc, mybir

F32 = mybir.dt.float32
F16 = mybir.dt.float16
MMDT = mybir.dt.float16
AF = mybir.ActivationFunctionType
OP = mybir.AluOpType

B = 8
NB = 8      # basis functions per activation
NS = 9      # d3 slabs per activation (d4 folded into weights)
NM = 12     # relu-cube slabs
O_OUT = 100
E = 256     # basis chunk width


def fold_weights(wb1, ws1, wb2, ws2, wb3, ws3, lb, lc):
    """Fold the 4th finite difference into conv weights:
    W'_s = (w_s - w_{s-1})/6 for s=0..8 (w_{-1} = w_{8} = 0)."""
    out = {}

    def d4fold(ws_k):  # ws_k: [O, C, NB] -> [NS, C, O]
        O_, C_, _ = ws_k.shape
        f = np.zeros((NS, C_, O_), np.float32)
        wt = np.transpose(ws_k, (2, 1, 0)) / 6.0  # [NB, C, O]
        f[0:NB] += wt
        f[1:NS] -= wt
        return f

    # W1: [90, 192]; per-ky 30-row block: rows 0:3 silu, 3:30 = 9 slabs x 3ch
    W1 = np.zeros((90, 3 * 64), np.float32)
    for kyi in range(3):
        for kxi in range(3):
            W1[kyi * 30 + 0:kyi * 30 + 3, kxi * 64:(kxi + 1) * 64] = wb1[:, :, kyi, kxi].T
            blk = d4fold(ws1[:, :, kyi, kxi].reshape(64, 3, NB))  # [9, 3, 64]
            W1[kyi * 30 + 3:kyi * 30 + 30, kxi * 64:(kxi + 1) * 64] = blk.reshape(27, 64)
    out['w1'] = W1.astype(MM_NP)

    # W2: [640, 1152]; rows 0:576 = 9 slabs x 64ch (slab-major), 576:640 silu
    W2 = np.zeros((640, 9 * 128), np.float32)
    for kyi in range(3):
        for kxi in range(3):
            t = kyi * 3 + kxi
            blk = d4fold(ws2[:, :, kyi, kxi].reshape(128, 64, NB))  # [9, 64, 128]
            W2[0:576, t * 128:(t + 1) * 128] = blk.reshape(576, 128)
            W2[576:640, t * 128:(t + 1) * 128] = wb2[:, :, kyi, kxi].T
    out['w2'] = W2.astype(MM_NP)

    # W3: [1280, 576]; rows 0:1152 = 9 slabs x 128ch, 1152:1280 silu
    W3 = np.zeros((1280, 9 * 64), np.float32)
    for kyi in range(3):
        for kxi in range(3):
            t = kyi * 3 + kxi
            blk = d4fold(ws3[:, :, kyi, kxi].reshape(64, 128, NB))  # [9, 128, 64]
            W3[0:1152, t * 64:(t + 1) * 64] = blk.reshape(1152, 64)
            W3[1152:1280, t * 64:(t + 1) * 64] = wb3[:, :, kyi, kxi].T
    out['w3'] = W3.astype(MM_NP)

    # WL: [128, 32*9*100]; rows (c + 64*par), col ((p2*9 + j)*100 + o)
    # j=0..7 spline basis (d4 stays on DVE for the linear layer), j=8 silu.
    # y-parity pairing: p2 = y2*8 + x, yx = (2*y2 + par)*8 + x.
    lc_r = lc.reshape(O_OUT, 64, 64, NB)
    lb_r = lb.reshape(O_OUT, 64, 64)
    WL = np.zeros((2, 64, 32, 9, O_OUT), np.float32)
    for par in range(2):
        for p2 in range(32):
            yx = (2 * (p2 // 8) + par) * 8 + (p2 % 8)
            for j in range(NB):
                WL[par, :, p2, j, :] = lc_r[:, :, yx, j].T / 6.0
            WL[par, :, p2, 8, :] = lb_r[:, :, yx].T
    out['wl'] = WL.reshape(128, 32 * 9 * O_OUT).astype(MM_NP)
    return out


def emit_basis(nc, bp, src_ap, P, d3_out_3d, bias_tiles, Ec=E, d2_pool=True,
               d1_pool=False):
    """Basis chunk: src [P, Ec] activations -> d3 slabs [P, NS, Ec] fp16 (3D AP).

    Engine split: ACT: RX, 12x T, 1x Square; DVE: cube, d3 (+d1/d2 unless
    routed to Pool via d1_pool/d2_pool for DVE-queue relief).
    """
    RX = bp.tile([P, Ec], F32, tag="rx")
    nc.scalar.activation(RX[:], src_ap, AF.Relu, bias=bias_tiles['c22'][0:P, :],
                         scale=-1.0)
    T12 = bp.tile([P, NM * Ec], F32, tag="t12")
    for m in range(NM):
        nc.scalar.activation(T12[:, m * Ec:(m + 1) * Ec], RX[:], AF.Relu,
                             bias=bias_tiles[m][0:P, :], scale=-2.5)
    S12 = bp.tile([P, NM * Ec], F32, tag="s12")
    nc.scalar.activation(S12[:], T12[:], AF.Square)
    C3 = bp.tile([P, NM * Ec], F32, tag="c3")
    nc.vector.tensor_tensor(C3[:], T12[:], S12[:], op=OP.mult)
    D1 = bp.tile([P, (NM - 1) * Ec], F32, tag="t12", name="D1")
    eng1 = nc.gpsimd if d1_pool else nc.vector
    eng1.tensor_tensor(D1[:], C3[:, 0:(NM - 1) * Ec], C3[:, Ec:NM * Ec], op=OP.subtract)
    D2 = bp.tile([P, (NM - 2) * Ec], F32, tag="s12", name="D2")
    eng2 = nc.gpsimd if d2_pool else nc.vector
    eng2.tensor_tensor(D2[:], D1[:, 0:(NM - 2) * Ec], D1[:, Ec:(NM - 1) * Ec], op=OP.subtract)
    nc.vector.tensor_tensor(
        d3_out_3d,
        D2[:, 0:NS * Ec].rearrange("p (s e) -> p s e", e=Ec),
        D2[:, Ec:(NS + 1) * Ec].rearrange("p (s e) -> p s e", e=Ec),
        op=OP.subtract)


def build_nc(dbg=()):
    nc = bacc.Bacc("TRN2", target_bir_lowering=False, debug=False, num_devices=8)
    x_ext = nc.declare_dram_parameter("x", [B, 3, 64, 64], F32, isOutput=False)
    w1_ext = nc.declare_dram_parameter("w1", [90, 192], MMDT, isOutput=False)
    w2_ext = nc.declare_dram_parameter("w2", [640, 1152], MMDT, isOutput=False)
    w3_ext = nc.declare_dram_parameter("w3", [1280, 576], MMDT, isOutput=False)
    wl_ext = nc.declare_dram_parameter("wl", [128, 28800], MMDT, isOutput=False)
    out_ext = nc.declare_dram_parameter("out", [B, O_OUT], F32, isOutput=True)

    dbg_exts = {}

    def dbg_tap(name, shape, dt=F32):
        if name in dbg:
            dbg_exts[name] = nc.declare_dram_parameter(f"dbg_{name}", shape, dt, isOutput=True)
            return dbg_exts[name]
        return None

    with tile.TileContext(nc) as tc, ExitStack() as ctx:
        persist = ctx.enter_context(tc.tile_pool(name="persist", bufs=1))
        wpool = ctx.enter_context(tc.tile_pool(name="wpool", bufs=1))
        bpool = ctx.enter_context(tc.tile_pool(name="bpool", bufs=2))

        bias_tiles = {}
        for m in range(NM):
            bt_m = wpool.tile([128, 1], F32, tag=f"bias_{m}", name=f"bias{m}")
            nc.gpsimd.memset(bt_m[:], float(11 - m))
            bias_tiles[m] = bt_m
        bt_c = wpool.tile([128, 1], F32, tag="bias_c22", name="biasc22")
        nc.gpsimd.memset(bt_c[:], 2.2)
        bias_tiles['c22'] = bt_c

        h1s = persist.tile([128, 4096], F16)   # L2 input: (ch, b<4|b>=4) x (sb, px)
        h2 = persist.tile([128, 2048], F32)    # L3 input: ch x (b, px)
        h3 = persist.tile([64, 512], F32)      # linear input: ch x (b, yx)

        # ================= L1 =================
        l1ch = nc.dram_tensor("l1ch", [30, B * 4096], MMDT)
        w1sb = wpool.tile([90, 192], MMDT)
        w2sb = wpool.tile([128, 5 * 1152], MMDT)
        w3sb = wpool.tile([128, 10 * 576], MMDT)
        zt = wpool.tile([30, 64], MMDT)
        with tc.tile_pool(name="l1pool", bufs=1) as l1p:
            # input + w1 first so the basis chain starts ASAP
            X1 = l1p.tile([128, 768], F32)
            for c in range(3):
                nc.sync.dma_start(
                    X1[:, c * 256:(c + 1) * 256],
                    x_ext.ap()[:, c, :, :].rearrange("b (g hh) w -> b g (hh w)", g=16))
            nc.sync.dma_start(w1sb[:], w1_ext.ap())
            nc.gpsimd.memset(zt[:], 0.0)

            sl1 = l1p.tile([128, 768], MMDT)
            nc.scalar.activation(sl1[:], X1[:], AF.Silu)

            D3_1 = l1p.tile([128, NS * 768], MMDT)
            d31_v = D3_1[:].rearrange("p (s e) -> p s e", s=NS)
            for ck in range(3):
                emit_basis(nc, bpool, X1[:, ck * E:(ck + 1) * E], 128,
                           d31_v[:, :, ck * E:(ck + 1) * E], bias_tiles,
                           d2_pool=False)

            # stage to channel-major DRAM: rows 0:3 silu ch, 3+s*3+c for slabs
            for s in range(NS):
                nc.sync.dma_start(
                    l1ch.ap()[3 + s * 3:3 + s * 3 + 3, :]
                        .rearrange("c (bg e) -> bg c e", e=256),
                    D3_1[:, s * 768:(s + 1) * 768]
                        .rearrange("p (c e) -> p c e", e=256))
            for c in range(3):
                nc.scalar.dma_start(
                    l1ch.ap()[c, :].rearrange("(bg e) -> bg e", e=256),
                    sl1[:, c * 256:(c + 1) * 256])
            # prefetch L2/L3 weights on the idle ACT queue
            for i in range(5):
                nc.scalar.dma_start(w2sb[:, i * 1152:(i + 1) * 1152],
                                    w2_ext.ap()[i * 128:(i + 1) * 128, :])
            for i in range(10):
                nc.scalar.dma_start(w3sb[:, i * 576:(i + 1) * 576],
                                    w3_ext.ap()[i * 128:(i + 1) * 128, :])

            # per image-pair (sb, sb+4) buns; img sb at bun pos 0 -> psum 0:64
            with tc.tile_pool(name="l1bun", bufs=2) as lbp, \
                    tc.tile_pool(name="pp1", bufs=2, space="PSUM") as pp1:
                for sb in range(4):
                    Bun1 = lbp.tile([90, 64 + 2 * 4096 + 64], MMDT, tag="bun1",
                                    name=f"Bun1_{sb}")
                    for kyi in range(3):
                        base = 64 + (1 - kyi) * 64
                        nc.sync.dma_start(
                            Bun1[kyi * 30:kyi * 30 + 30, base:base + 2 * 4096]
                                .rearrange("p (b2 e) -> p b2 e", b2=2),
                            l1ch.ap()[:, :].rearrange("p (b e) -> p b e", b=8)[
                                :, sb::4, :])
                    # boundary rows: ky=0 block box-row 0; ky=2 block box-row 63
                    for bi in range(2):
                        nc.sync.dma_start(
                            Bun1[0:30, 64 + bi * 4096: 64 + bi * 4096 + 64], zt[:])
                        nc.sync.dma_start(
                            Bun1[60:90, 64 + bi * 4096 + 63 * 64: 64 + bi * 4096 + 64 * 64],
                            zt[:])

                    bun1_v = Bun1[:, 64:64 + 2 * 4096].rearrange(
                        "p (b r w) -> p b r w", b=2, w=64)
                    for grp in range(2):
                        pss = [pp1.tile([128, 512], F32, tag=f"ps1_{i}",
                                        name=f"ps1_{sb}_{grp}_{i}") for i in range(4)]
                        for ti, kxi in enumerate([1, 0, 2]):
                            for ci in range(4):
                                hb = grp * 4 + ci
                                ps = pss[ci]
                                for half in range(2):
                                    if kxi == 0:
                                        mv = bun1_v[:, half, hb * 8:hb * 8 + 8, 0:63]
                                        ov = ps[half * 64:half * 64 + 64, :].rearrange(
                                            "p (r w) -> p r w", w=64)[:, :, 1:64]
                                    elif kxi == 1:
                                        mv = bun1_v[:, half, hb * 8:hb * 8 + 8, :]
                                        ov = ps[half * 64:half * 64 + 64, :]
                                    else:
                                        mv = bun1_v[:, half, hb * 8:hb * 8 + 8, 1:64]
                                        ov = ps[half * 64:half * 64 + 64, :].rearrange(
                                            "p (r w) -> p r w", w=64)[:, :, 0:63]
                                    nc.tensor.matmul(ov, w1sb[:, kxi * 64:(kxi + 1) * 64],
                                                     mv, start=(ti == 0), stop=(ti == 2))
                        for ci in range(4):
                            hb = grp * 4 + ci
                            pv = pss[ci][:].rearrange("p (hp r2 wp c2) -> p hp wp r2 c2",
                                                      hp=4, r2=2, wp=32, c2=2)
                            nc.vector.tensor_reduce(
                                h1s[:, sb * 1024 + hb * 128: sb * 1024 + (hb + 1) * 128]
                                    .rearrange("p (hp wp) -> p hp wp", wp=32),
                                pv, mybir.AxisListType.XY, OP.max, opt_input=False)
        if (t := dbg_tap('h1s', [128, 4096], F16)) is not None:
            nc.sync.dma_start(t.ap(), h1s[:])

        # ================= L2 =================
        # sub-batch sb: images (sb, sb+4) = h1s cols [sb*1024, (sb+1)*1024)
        with tc.tile_pool(name="l2t", bufs=2) as l2t, \
                tc.tile_pool(name="l2s", bufs=2) as l2s, \
                tc.tile_pool(name="pp2", bufs=2, space="PSUM") as pp2:
            for sb in range(4):
                T2 = l2t.tile([128, 5 * 2048], MMDT, tag="t2", name=f"T2_{sb}")
                # silu: img sb+4 (h1s rows 64:128) writes lanes 64:128 directly
                nc.scalar.activation(T2[64:128, 4 * 2048 + 1024:4 * 2048 + 2048],
                                     h1s[64:128, sb * 1024:(sb + 1) * 1024], AF.Silu)
                Ts2a = l2s.tile([64, 1024], MMDT, tag="ts2a", name=f"Ts2a_{sb}")
                nc.scalar.activation(Ts2a[:], h1s[0:64, sb * 1024:(sb + 1) * 1024], AF.Silu)
                nc.sync.dma_start(T2[64:128, 4 * 2048:4 * 2048 + 1024], Ts2a[:])

                D3b = l2s.tile([128, NS * 1024], MMDT, tag="d3", name=f"D3_{sb}")
                d3v = D3b[:].rearrange("p (s q) -> p s q", s=NS)
                for ck in range(4):
                    emit_basis(nc, bpool,
                               h1s[:, sb * 1024 + ck * E: sb * 1024 + (ck + 1) * E],
                               128, d3v[:, :, ck * E:(ck + 1) * E], bias_tiles,
                               d1_pool=(ck % 2 == 1))
                # scatter into T2: slab s -> col-block s//2, rows (s%2)*64+c,
                # col ph*1024 (6 batched DMAs per sub-batch)
                for ph in range(2):
                    for s1 in range(2):
                        nc.sync.dma_start(
                            T2[s1 * 64:s1 * 64 + 64, :]
                                .rearrange("p (s2 q) -> p s2 q", s2=5)[
                                    :, 0:4, ph * 1024:(ph + 1) * 1024],
                            d3v[ph * 64:(ph + 1) * 64, s1:8:2, :])
                    nc.sync.dma_start(
                        T2[0:64, 4 * 2048 + ph * 1024:4 * 2048 + (ph + 1) * 1024],
                        d3v[ph * 64:(ph + 1) * 64, 8, :])

                t2v = T2[:].rearrange("p (kt b h w) -> p kt b h w", kt=5, b=2, w=32)
                taps = [(0, 1, 1)] + [(kt, kyi, kxi) for kt in range(5)
                                      for kyi in range(3) for kxi in range(3)
                                      if (kt, kyi, kxi) != (0, 1, 1)]
                n_taps = len(taps)
                pss = [pp2.tile([128, 512], F32, tag=f"ps2_{i}", name=f"ps2_{sb}_{i}")
                       for i in range(4)]
                for tapi, (kt, kyi, kxi) in enumerate(taps):
                    for ci in range(4):
                        ii, half = ci // 2, ci % 2
                        h0 = half * 16
                        ps = pss[ci]
                        r_lo = max(0, 1 - kyi - h0)
                        r_hi = min(16, 33 - h0 - kyi)
                        w_lo = 1 if kxi == 0 else 0
                        w_hi = 31 if kxi == 2 else 32
                        in_row = h0 + r_lo + kyi - 1
                        in_col = w_lo + kxi - 1
                        mv = t2v[:, kt, ii, in_row:in_row + (r_hi - r_lo),
                                 in_col:in_col + (w_hi - w_lo)]
                        ov = ps[:].rearrange("p (r w) -> p r w", w=32)[
                            :, r_lo:r_hi, w_lo:w_hi]
                        nc.tensor.matmul(
                            ov, w2sb[:, kt * 1152 + (kyi * 3 + kxi) * 128:
                                     kt * 1152 + (kyi * 3 + kxi + 1) * 128],
                            mv, start=(tapi == 0), stop=(tapi == n_taps - 1))
                for ci in range(4):
                    ii, half = ci // 2, ci % 2
                    b = sb + 4 * ii
                    pv = pss[ci][:].rearrange("p (hp r2 wp c2) -> p hp wp r2 c2",
                                              hp=8, r2=2, wp=16, c2=2)
                    nc.vector.tensor_reduce(
                        h2[:, b * 256 + half * 128: b * 256 + (half + 1) * 128]
                            .rearrange("p (hp wp) -> p hp wp", wp=16),
                        pv, mybir.AxisListType.XY, OP.max, opt_input=False)
        if (t := dbg_tap('h2', [128, 2048])) is not None:
            nc.sync.dma_start(t.ap(), h2[:])

        # prefetch wl during L3 (pool opened here: reuses bytes freed by L1/L2)
        wlp = ctx.enter_context(tc.tile_pool(name="wlpool", bufs=1))
        wlsb = wlp.tile([128, 28800], MMDT)
        for i in range(4):
            nc.sync.dma_start(wlsb[:, i * 7200:(i + 1) * 7200],
                              wl_ext.ap()[:, i * 7200:(i + 1) * 7200])

        # ================= L3 (+ interleaved linear basis halves) ==========
        # linear: h3r rows (c + 64*par), cols (b, y2); half hf = imgs 4hf..4hf+3
        h3r = persist.tile([128, 256], F32)
        sl3 = persist.tile([128, 256], MMDT)
        D3L = persist.tile([128, NS * 256], MMDT)
        D4L = persist.tile([128, NB * 256], MMDT)
        h3v = h3[:].rearrange("p (b y2 par x) -> p b y2 par x", b=8, y2=4, par=2)
        d3lv = D3L[:].rearrange("p (s q) -> p s q", s=NS)

        def emit_lin_half(hf):
            for par in range(2):
                for bi in range(4):
                    b = 4 * hf + bi
                    nc.sync.dma_start(
                        h3r[par * 64:par * 64 + 64,
                            hf * 128 + bi * 32:hf * 128 + (bi + 1) * 32]
                            .rearrange("p (y2 x) -> p y2 x", y2=4),
                        h3v[:, b, :, par, :])
            nc.scalar.activation(sl3[:, hf * 128:(hf + 1) * 128],
                                 h3r[:, hf * 128:(hf + 1) * 128], AF.Silu)
            emit_basis(nc, bpool, h3r[:, hf * 128:(hf + 1) * 128], 128,
                       d3lv[:, :, hf * 128:(hf + 1) * 128],
                       bias_tiles, Ec=128, d2_pool=False)
            nc.vector.tensor_tensor(
                D4L[:].rearrange("p (j q) -> p j q", j=NB)[:, :, hf * 128:(hf + 1) * 128],
                d3lv[:, 0:NB, hf * 128:(hf + 1) * 128],
                d3lv[:, 1:NS, hf * 128:(hf + 1) * 128],
                op=OP.subtract)

        # sub-batch sb: images (2sb, 2sb+1) = h2 cols [sb*512, (sb+1)*512)
        with tc.tile_pool(name="l3t", bufs=2) as l3t, \
                tc.tile_pool(name="pp3", bufs=2, space="PSUM") as pp3:
            for sb in range(4):
                T3 = l3t.tile([128, 10 * 512], MMDT, tag="t3", name=f"T3_{sb}")
                for ph in range(2):
                    b = 2 * sb + ph
                    nc.scalar.activation(T3[:, 9 * 512 + ph * 256:9 * 512 + (ph + 1) * 256],
                                         h2[:, b * 256:(b + 1) * 256], AF.Silu)
                    # d3 writes strided straight into T3 slab blocks
                    emit_basis(nc, bpool, h2[:, b * 256:(b + 1) * 256], 128,
                               T3[:].rearrange("p (s e) -> p s e", e=512)[
                                   :, 0:NS, ph * 256:(ph + 1) * 256],
                               bias_tiles, d1_pool=(ph == 1))
                if sb == 3:
                    emit_lin_half(0)   # after sb3 basis: ACT queue no longer blocked

                t3v = T3[:].rearrange("p (kt b h w) -> p kt b h w", kt=10, b=2, w=16)
                taps = [(0, 1, 1)] + [(kt, kyi, kxi) for kt in range(10)
                                      for kyi in range(3) for kxi in range(3)
                                      if (kt, kyi, kxi) != (0, 1, 1)]
                n_taps = len(taps)
                ps = pp3.tile([64, 512], F32, tag="ps3", name=f"ps3_{sb}")
                for tapi, (kt, kyi, kxi) in enumerate(taps):
                    r_lo = max(0, 1 - kyi)
                    r_hi = min(16, 17 - kyi)
                    w_lo = 1 if kxi == 0 else 0
                    w_hi = 15 if kxi == 2 else 16
                    mv = t3v[:, kt, :, r_lo + kyi - 1:r_hi + kyi - 1,
                             w_lo + kxi - 1:w_lo + kxi - 1 + (w_hi - w_lo)]
                    ov = ps[:].rearrange("p (b r w) -> p b r w", b=2, w=16)[
                        :, :, r_lo:r_hi, w_lo:w_hi]
                    nc.tensor.matmul(
                        ov, w3sb[:, kt * 576 + (kyi * 3 + kxi) * 64:
                                 kt * 576 + (kyi * 3 + kxi + 1) * 64],
                        mv, start=(tapi == 0), stop=(tapi == n_taps - 1))
                pv = ps[:].rearrange("p (b hp r2 wp c2) -> p b hp wp r2 c2",
                                     b=2, hp=8, r2=2, wp=8, c2=2)
                for ph in range(2):
                    b = 2 * sb + ph
                    nc.vector.tensor_reduce(
                        h3[:, b * 64:(b + 1) * 64].rearrange("p (hp wp) -> p hp wp", wp=8),
                        pv[:, ph], mybir.AxisListType.XY, OP.max, opt_input=False)
            emit_lin_half(1)
        if (t := dbg_tap('h3', [64, 512])) is not None:
            nc.sync.dma_start(t.ap(), h3[:])
        if (t := dbg_tap('d4l', [128, NB * 256], F16)) is not None:
            nc.sync.dma_start(t.ap(), D4L[:])
        if (t := dbg_tap('sl3', [128, 256], F16)) is not None:
            nc.sync.dma_start(t.ap(), sl3[:])
        if (t := dbg_tap('h3r', [128, 256])) is not None:
            nc.sync.dma_start(t.ap(), h3r[:])

        # ================= Linear matmuls =================
        with tc.tile_pool(name="ppl", bufs=1, space="PSUM") as plin:
            psl = plin.tile([B, O_OUT], F32)
            d4l_v = D4L[:].rearrange("p (j b y2) -> p j b y2", j=NB, b=B)
            sl3_v = sl3[:].rearrange("p (b y2) -> p b y2", b=B)
            for p2 in range(32):
                for j in range(NB):
                    nc.tensor.matmul(
                        psl[:], d4l_v[:, j, :, p2],
                        wlsb[:, (p2 * 9 + j) * O_OUT:(p2 * 9 + j + 1) * O_OUT],
                        start=(p2 == 0 and j == 0), stop=False)
                nc.tensor.matmul(
                    psl[:], sl3_v[:, :, p2],
                    wlsb[:, (p2 * 9 + 8) * O_OUT:(p2 * 9 + 9) * O_OUT],
                    start=False, stop=(p2 == 31))
            osb = persist.tile([B, O_OUT], F32)
            nc.vector.tensor_copy(osb[:], psl[:])
            nc.sync.dma_start(out_ext.ap(), osb[:])

    nc.compile()
    return nc

# ===================================================================== runner
from concourse.bass_utils import run_bass_kernel_spmd

_NC_CACHE = {}


def _get_nc():
    if 'nc' not in _NC_CACHE:
        _NC_CACHE['nc'] = build_nc(dbg=())
    return _NC_CACHE['nc']


def kernel(x, wb1, ws1, wb2, ws2, wb3, ws3, lb, lc):
    """Full-input entry point: x [64,3,64,64] f32 -> out [64,100] f32.
    Shards the batch over 8 NeuronCores (8 samples each), replicating weights."""
    x = np.ascontiguousarray(np.asarray(x, dtype=np.float32))
    w = fold_weights(np.asarray(wb1, np.float32), np.asarray(ws1, np.float32),
                     np.asarray(wb2, np.float32), np.asarray(ws2, np.float32),
                     np.asarray(wb3, np.float32), np.asarray(ws3, np.float32),
                     np.asarray(lb, np.float32), np.asarray(lc, np.float32))
    nc = _get_nc()
    in_maps = [{'x': x[i * B:(i + 1) * B], **w} for i in range(8)]
    res = run_bass_kernel_spmd(nc, in_maps, core_ids=list(range(8)))
    return np.concatenate([res.results[i]['out'] for i in range(8)], axis=0)
